# revision 1
# baseline (speedup 1.0000x reference)
"""Trainium2 Bass kernel for nn_Attention_89172110999574.

Strategy (8 NeuronCores, data parallel - 1 batch element per core):
  - All matmul operands bf16 (1 cyc/row on PE); PSUM stays f32.
  - Scores computed TRANSPOSED (ST[j,i] = k_j . q_i); softmax scale folded
    into Wq at load.
  - Relative-position bias is block-Toeplitz; two per-head strip tables are
    built once via a handful of large-elem DMAs (DRAM bounce for the
    partition reshuffle):
      * ms  (bf16):  exp(bias) factors for the exact-exp path (jt 0-3)
      * msa (int16): bias*A + B Schraudolph addends for the fast path
    exp(s+b) = exp(s)*exp(b) on the A path (ACT exp + DVE/Pool
    tensor_tensor multiply at 2x bf16 rate), and
    bf16_bits(exp(s+b)) ~= s*A + (b*A + B) on the V path - a single DVE
    op (Schraudolph's trick in bf16, max ~3% weight ripple, cancels in
    the softmax ratio; validated 4e-3 end-to-end).
  - attn@V uses exp-scores as stationary -> [i, dv+1] tiles with the
    softmax denominator in the last column.  Each PSUM accumulation group
    runs as 8 consecutive matmuls (interleaved groups in one bank
    misaccumulate on hardware).
  - normalize+gelu fused: ACT Gelu with per-partition scale = 1/den
    (reciprocals batched 4-wide on DVE) reading straight from PSUM.
  - BatchNorm affine folded into Wo/bias rows; per-i-tile tail:
    transpose via PE, final contraction, store.
"""

import os
import sys

import numpy as np

for _p in ("/opt/trn_rl_repo", "/root/.axon_site/_ro/trn_rl_repo"):
    if os.path.isdir(_p) and _p not in sys.path:
        sys.path.insert(0, _p)

import concourse.bass as bass
import concourse.tile as tile
from concourse import mybir
from concourse.bass_utils import run_bass_kernel_spmd
from concourse.masks import make_identity

N = 1024          # tokens per batch (32*32)
D = 256           # model dim
H = 8             # heads
DK = 32           # head dim (qk)
DV = 64           # head dim (v)
DOUT = 256        # output dim
NCORES = 8
FM = 32           # fmap
SCALE = float(DK) ** -0.5          # 1/sqrt(32)
BN_C = float(1.0 / np.sqrt(1.0 + 1e-5))
SCH_A = float(2 ** 7 / np.log(2.0))        # Schraudolph bf16 scale
SCH_B = float(127 * 2 ** 7 - 7.4 + 0.5)    # bias - minimax + trunc comp
F32 = mybir.dt.float32
BF16 = mybir.dt.bfloat16
I16 = mybir.dt.int16
AF = mybir.ActivationFunctionType
ALU = mybir.AluOpType

# jt 0..3: exact exp on ACT (+ bias multiply); jt 4..7: Schraudolph on DVE
N_A = 4
POOL_BIAS_JT = (0, 1, 2)       # A-tiles whose bias multiply runs on Pool
# strip-table u ranges (u0 = 31-4*jt, slice [u0, u0+32))
MS_U0, MS_UN = 19, 44          # A tiles: u in [19, 62]
MSA_U0, MSA_UN = 3, 44         # V tiles: u in [3, 46]


def build_nc():
    nc = bass.Bass("TRN2", target_bir_lowering=False, debug=False)

    x = nc.dram_tensor("x", [N, D], F32, kind="ExternalInput").ap()
    wq = nc.dram_tensor("wq", [D, H * DK], F32, kind="ExternalInput").ap()
    wk = nc.dram_tensor("wk", [D, H * DK], F32, kind="ExternalInput").ap()
    wv = nc.dram_tensor("wv", [D, H * DV], F32, kind="ExternalInput").ap()
    wo = nc.dram_tensor("wo", [H * DV, DOUT], F32, kind="ExternalInput").ap()
    pe = nc.dram_tensor("pe", [N, H], F32, kind="ExternalInput").ap()
    bo = nc.dram_tensor("bo", [DOUT], F32, kind="ExternalInput").ap()
    gam = nc.dram_tensor("gam", [DOUT], F32, kind="ExternalInput").ap()
    bet = nc.dram_tensor("bet", [DOUT], F32, kind="ExternalInput").ap()
    out = nc.dram_tensor("out", [N, DOUT], F32, kind="ExternalOutput").ap()

    # DRAM bounce buffers for the strip tables: [a, h, s] flipped rows
    wrowd = nc.dram_tensor("wrowd", [32, 8, 63], BF16).ap()
    wrowda = nc.dram_tensor("wrowda", [32, 8, 63], I16).ap()

    with tile.TileContext(nc) as tc:
        with (
            tc.tile_pool(name="const", bufs=1) as constp,
            tc.tile_pool(name="big", bufs=1) as bigp,
            tc.tile_pool(name="xin", bufs=3) as xinp,
            tc.tile_pool(name="exps", bufs=4) as expp,
            tc.tile_pool(name="esb", bufs=1) as esbp,
            tc.tile_pool(name="small", bufs=2) as smallp,
            tc.tile_pool(name="yout", bufs=3) as youtp,
            tc.tile_pool(name="ps1", bufs=2, space="PSUM") as ps1p,
            tc.tile_pool(name="st", bufs=2, space="PSUM") as ps2p,
            tc.tile_pool(name="po", bufs=1, space="PSUM") as pop,
        ):
            # ------------- input / weight DMAs first (fabric order) -----
            xa = bigp.tile([128, 8, 256], F32)
            for c in range(2):
                nc.sync.dma_start(
                    out=xa[:, 4 * c:4 * (c + 1), :],
                    in_=bass.AP(tensor=x.tensor, offset=4 * c * 128 * 256,
                                ap=[[256, 128], [128 * 256, 4], [1, 256]]))
            wstg_q = xinp.tile([128, 2, 256], F32, tag="wstg2", bufs=2)
            nc.scalar.dma_start(
                out=wstg_q,
                in_=bass.AP(tensor=wq.tensor, offset=0,
                            ap=[[256, 128], [128 * 256, 2], [1, 256]]))
            wstg_k = xinp.tile([128, 2, 256], F32, tag="wstg2", bufs=2)
            nc.scalar.dma_start(
                out=wstg_k,
                in_=bass.AP(tensor=wk.tensor, offset=0,
                            ap=[[256, 128], [128 * 256, 2], [1, 256]]))
            wstg_v = xinp.tile([128, 2, 512], F32, tag="wstgv", bufs=1)
            nc.scalar.dma_start(
                out=wstg_v,
                in_=bass.AP(tensor=wv.tensor, offset=0,
                            ap=[[512, 128], [128 * 512, 2], [1, 512]]))
            e_sb = smallp.tile([32, 32, 8], F32, tag="e_sb")
            nc.sync.dma_start(
                out=e_sb,
                in_=bass.AP(tensor=pe.tensor, offset=0,
                            ap=[[32 * H, 32], [H, 32], [1, 32 * H // 32]]),
            )

            # ---------------- constants -------------------------------
            identb = constp.tile([128, 128], BF16)
            make_identity(nc, identb)
            identf = constp.tile([128, 128], F32)
            nc.gpsimd.tensor_copy(identf, identb)

            # ---------------- strip tables ----------------------------
            # 2a) mult table: ee = exp(pe/scale) bf16
            ee = smallp.tile([32, 32, 8], BF16, tag="ee")
            nc.scalar.activation(ee, e_sb, AF.Exp, scale=1.0 / SCALE)
            # 2b) additive Schraudolph table: eea = pe/scale*A + B int16
            eea = smallp.tile([32, 32, 8], I16, tag="eea")
            nc.scalar.activation(eea, e_sb, AF.Copy,
                                 scale=SCH_A / SCALE, bias=SCH_B)
            # 3) s-flip both: wrow[a, h, s] = tab[a, |s-31|, h]
            wrow = smallp.tile([32, 8, 63], BF16, tag="wrow")
            wrowa = smallp.tile([32, 8, 63], I16, tag="wrowa")
            for wdst, wsrc in ((wrow, ee), (wrowa, eea)):
                nc.gpsimd.tensor_copy(
                    wdst[:, :, 0:31],
                    bass.AP(tensor=wsrc.tensor, offset=wsrc.offset + 31 * 8,
                            ap=[wsrc.ap[0], [1, 8], [-8, 31]]),
                )
                nc.gpsimd.tensor_copy(
                    wdst[:, :, 31:63],
                    bass.AP(tensor=wsrc.tensor, offset=wsrc.offset,
                            ap=[wsrc.ap[0], [1, 8], [8, 32]]),
                )
            # 4) DRAM bounce + gather (positive strides; wrow palindromic
            #    in s, so the gathered ci axis comes out reversed)
            nc.sync.dma_start(out=wrowd, in_=wrow)
            nc.scalar.dma_start(out=wrowda, in_=wrowa)
            # ms[(g,cj), u-U0, h, ci'] = tab_h[|u-31-g|, |ci-cj|]
            ms = bigp.tile([128, MS_UN, H, 32], BF16)
            msa = bigp.tile([128, MSA_UN, H, 32], I16)
            with tc.tile_pool(name="uw", bufs=1) as uwp:
                uwsb = uwp.tile([32, 32, H, 32], BF16)
                uwsba = uwp.tile([32, 32, H, 32], I16)
                nc.sync.dma_start(
                    out=uwsb,
                    in_=bass.AP(tensor=wrowd.tensor, offset=0,
                                ap=[[1, 32], [504, 32], [63, 8], [1, 32]]),
                )
                nc.scalar.dma_start(
                    out=uwsba,
                    in_=bass.AP(tensor=wrowda.tensor, offset=0,
                                ap=[[1, 32], [504, 32], [63, 8], [1, 32]]),
                )
                # 5) u-expansion: dst[(g,cj), u] = uwsb[cj, |u-31-g|]
                engs = (nc.sync, nc.scalar, nc.gpsimd)
                ei = 0
                for src, dst, u0t, un in ((uwsb, ms, MS_U0, MS_UN),
                                          (uwsba, msa, MSA_U0, MSA_UN)):
                    for g in range(4):
                        # upper: u in [31+g, u0t+un), a = u-31-g ascending
                        ua, ub = 31 + g, u0t + un
                        engs[ei % 3].dma_start(
                            out=dst[32 * g:32 * (g + 1), ua - u0t:ub - u0t,
                                    :, :],
                            in_=src[:, 0:ub - ua, :, :],
                        )
                        ei += 1
                        # lower: u in [u0t, 31+g), a = 31+g-u descending
                        la, lb = u0t, 31 + g
                        amax = 31 + g - u0t
                        engs[ei % 3].dma_start(
                            out=dst[32 * g:32 * (g + 1), 0:lb - la, :, :],
                            in_=bass.AP(tensor=src.tensor,
                                        offset=src.offset + amax * 256,
                                        ap=[src.ap[0], [-256, lb - la],
                                            [1, 256]]),
                        )
                        ei += 1

            # ---------------- BN affine rows ---------------------------
            g2b = constp.tile([128, DOUT], F32)
            b2b = constp.tile([128, DOUT], F32)
            tmpb = constp.tile([128, DOUT], F32)
            nc.sync.dma_start(
                out=g2b, in_=bass.AP(tensor=gam.tensor, offset=0,
                                     ap=[[0, 128], [1, DOUT]]))
            nc.sync.dma_start(
                out=b2b, in_=bass.AP(tensor=bet.tensor, offset=0,
                                     ap=[[0, 128], [1, DOUT]]))
            nc.sync.dma_start(
                out=tmpb, in_=bass.AP(tensor=bo.tensor, offset=0,
                                      ap=[[0, 128], [1, DOUT]]))
            wstg_o = xinp.tile([128, 4, 256], F32, tag="wstgo", bufs=1)
            nc.sync.dma_start(
                out=wstg_o,
                in_=bass.AP(tensor=wo.tensor, offset=0,
                            ap=[[256, 128], [128 * 256, 4], [1, 256]]))
            nc.scalar.mul(g2b, g2b, BN_C)
            nc.vector.tensor_mul(tmpb, tmpb, g2b)
            nc.vector.tensor_add(b2b, b2b, tmpb)

            # ---------------- weights (convert to bf16) ----------------
            wq_sb = constp.tile([128, 2, 256], BF16)
            wk_sb = constp.tile([128, 2, 256], BF16)
            wv_sb = constp.tile([128, 2, 512], BF16)
            wo_sb = constp.tile([128, 4, 256], BF16)
            nc.scalar.mul(wq_sb, wstg_q, SCALE)   # fold softmax scale
            nc.gpsimd.tensor_copy(wk_sb, wstg_k)
            nc.gpsimd.tensor_copy(wv_sb, wstg_v)
            # fold BN gamma*c into Wo columns (broadcast g2b over kt)
            nc.vector.tensor_mul(
                wo_sb, wstg_o,
                bass.AP(tensor=g2b.tensor, offset=g2b.offset,
                        ap=[g2b.ap[0], [0, 4], [1, 256]]))

            def _copy(i, dst, src):
                e = (nc.scalar.copy, nc.vector.tensor_copy)[i % 2]
                e(dst, src)

            # ---------------- phase A: x -> xT (bf16) ------------------
            xT = bigp.tile([128, 2, N], BF16)

            def _phase_a(nts):
                for nt in nts:
                    pst = ps1p.tile([128, 512], F32, tag="ps1")
                    for dt in range(2):
                        nc.tensor.transpose(pst[:, 128 * dt:128 * (dt + 1)],
                                            xa[:, nt, 128 * dt:128 * (dt + 1)],
                                            identf)
                    _copy(nt, xT[:, 0, 128 * nt:128 * (nt + 1)],
                          pst[:, 0:128])
                    _copy(nt + 1, xT[:, 1, 128 * nt:128 * (nt + 1)],
                          pst[:, 128:256])

            qT = bigp.tile([128, 2, N], BF16)
            kT = bigp.tile([128, 2, N], BF16)
            va = bigp.tile([128, 8, H, 65], BF16)
            nc.scalar.activation(va[:, :, :, 64:65], identb[:, 0:64],
                                 AF.Copy, bias=1.0, scale=0.0)
            ci_ = 0

            def _qk(mt, ics=(0, 1)):
                nonlocal ci_
                for dst_sb, w_sb in ((qT, wq_sb), (kT, wk_sb)):
                    for ic in ics:
                        ps = ps1p.tile([128, 512], F32, tag="ps1")
                        for kt in range(2):
                            nc.tensor.matmul(
                                ps,
                                w_sb[:, kt, 128 * mt:128 * (mt + 1)],
                                xT[:, kt, 512 * ic:512 * (ic + 1)],
                                start=(kt == 0), stop=(kt == 1),
                            )
                        _copy(ci_, dst_sb[:, mt, 512 * ic:512 * (ic + 1)], ps)
                        ci_ += 1

            # tokens 0-511 transposed first -> q/k ic=0 can start at once
            _phase_a(range(4))
            _qk(0, ics=(0,))
            _phase_a(range(4, 8))
            _qk(0, ics=(1,))
            for jt in range(8):
                ps = ps1p.tile([128, 512], F32, tag="ps1")
                for kt in range(2):
                    nc.tensor.matmul(
                        ps,
                        xT[:, kt, 128 * jt:128 * (jt + 1)],
                        wv_sb[:, kt, :],
                        start=(kt == 0), stop=(kt == 1),
                    )
                psr = ps.rearrange("p (h v) -> p h v", v=64)
                _copy(ci_, va[:, jt, :, 0:64], psr)
                ci_ += 1
            _qk(1)

            # ---------------- phase C: attention -----------------------
            # g_all[i-part, it, h, dv] collects gelu(attn/den), bf16
            g_all = bigp.tile([128, 8, H, DV], BF16)
            gtt = bigp.tile([128, 4, 8, 128], BF16)

            def _tail_transpose(blk):
                # gtt[:, blk, it, :] = g_all[:, it, 2blk:2blk+2, :]^T
                for it in range(8):
                    pst = ps1p.tile([128, 512], F32, tag="ps1")
                    pstb = pst[:, 0:64].bitcast(BF16)
                    nc.tensor.transpose(
                        pstb,
                        g_all[:, it, 2 * blk:2 * blk + 2, :], identb)
                    nc.vector.tensor_copy(gtt[:, blk, it, :], pstb)

            def _attnv_group(h, it, esbbs, pos):
                for jt in range(8):
                    nc.tensor.matmul(
                        pos[it // 4][:, it % 4, :],
                        esbbs[jt][:, 128 * it:128 * (it + 1)],
                        va[:, jt, h, :],
                        start=(jt == 0), stop=(jt == 7),
                    )

            def _norm_gelu(h, half, pos, rcp):
                den = pos[half][:, :, 64:65]
                nc.vector.reciprocal(
                    rcp[:, 4 * half:4 * half + 4],
                    bass.AP(tensor=den.tensor, offset=den.offset,
                            ap=[den.ap[0], [65, 4]]))
                for it in range(4 * half, 4 * half + 4):
                    nc.scalar.activation(
                        g_all[:, it, h, :],
                        pos[half][:, it % 4, 0:64],
                        AF.Gelu, scale=rcp[:, it:it + 1])

            # attnV/normalize are software-pipelined one head behind the
            # score/exp stream so the attnV matmul groups fill PE stalls
            # between score matmuls (keeps PE p-state ramped).
            prev = None
            for h in range(H):
                mtk = h // 4
                pb = 32 * (h % 4)
                po0 = pop.tile([128, 4, 65], F32, tag="po0")
                po1 = pop.tile([128, 4, 65], F32, tag="po1")
                pos = (po0, po1)
                esbbs = [None] * 8
                jt_order = ((0, 1, 2, 3, 4, 5, 6, 7) if h == 0
                            else (0, 4, 1, 5, 2, 6, 3, 7))
                for step, jt in enumerate(jt_order):
                    u0 = 31 - 4 * jt
                    ps = ps2p.tile([128, 1024], F32, tag="st")
                    for ic in range(2):
                        nc.tensor.matmul(
                            ps[:, 512 * ic:512 * (ic + 1)],
                            kT[pb:pb + 32, mtk, 128 * jt:128 * (jt + 1)],
                            qT[pb:pb + 32, mtk, 512 * ic:512 * (ic + 1)],
                            start=True, stop=True,
                            tile_position=(pb, 0),
                        )
                    if jt < N_A:
                        # exact exp on ACT, bias multiply (DVE 2x / Pool)
                        es = expp.tile([128, 1024], BF16, tag="es")
                        nc.scalar.activation(es, ps, AF.Exp)
                        esb = esbp.tile([128, 1024], BF16, tag="esb", bufs=8)
                        msl = ms[:, u0 - MS_U0:u0 - MS_U0 + 32, h, :]
                        msr = bass.AP(tensor=msl.tensor,
                                      offset=msl.offset + 31,
                                      ap=[msl.ap[0], msl.ap[1], [-1, 32]])
                        beng = nc.gpsimd if jt in POOL_BIAS_JT else nc.vector
                        beng.tensor_tensor(esb, es, msr, ALU.mult)
                        esbbs[jt] = esb
                    else:
                        # fused Schraudolph exp+bias on DVE:
                        # bits_i16 = ps*A + (b/scale*A + B) -> bitcast bf16
                        esb = esbp.tile([128, 1024], I16, tag="esbi", bufs=8)
                        msl = msa[:, u0 - MSA_U0:u0 - MSA_U0 + 32, h, :]
                        msr = bass.AP(tensor=msl.tensor,
                                      offset=msl.offset + 31,
                                      ap=[msl.ap[0], msl.ap[1], [-1, 32]])
                        nc.vector.scalar_tensor_tensor(
                            esb, ps, SCH_A, msr, ALU.mult, ALU.add)
                        esbbs[jt] = esb.bitcast(BF16)
                    # head h-1's attnV/normalize, software-pipelined into
                    # this head's score/exp stream: groups land early and
                    # paired; gelus only once their attnV half has closed
                    # (avoids ACT head-of-line stalls)
                    if prev is not None:
                        pesb, ppos, prcp = prev
                        if step == 0:
                            for g_ in range(4):
                                _attnv_group(h - 1, g_, pesb, ppos)
                        elif step == 1:
                            _norm_gelu(h - 1, 0, ppos, prcp)
                            for g_ in range(4, 8):
                                _attnv_group(h - 1, g_, pesb, ppos)
                        elif step == 2:
                            _norm_gelu(h - 1, 1, ppos, prcp)
                        elif step == 3 and h % 2 == 0:
                            _tail_transpose(h // 2 - 1)
                rcp = smallp.tile([128, 8], F32, tag="rcp", bufs=3)
                prev = (esbbs, pos, rcp)
            # drain last head
            pesb, ppos, prcp = prev
            def _drain_it(it):
                pst = ps1p.tile([128, 512], F32, tag="ps1")
                pstb = pst[:, 0:64].bitcast(BF16)
                nc.tensor.transpose(pstb, g_all[:, it, 6:8, :], identb)
                nc.vector.tensor_copy(gtt[:, 3, it, :], pstb)
                ps = ps1p.tile([128, 512], F32, tag="ps1")
                for kt in range(4):
                    nc.tensor.matmul(
                        ps[:, 0:256],
                        gtt[:, kt, it, :],
                        wo_sb[:, kt, :],
                        start=(kt == 0), stop=(kt == 3),
                    )
                yt = youtp.tile([128, DOUT], F32, tag="yt", bufs=8)
                nc.vector.tensor_add(yt, ps[:, 0:256], b2b)
                eng = nc.sync if it % 2 == 0 else nc.scalar
                eng.dma_start(out=out[128 * it:128 * (it + 1), :], in_=yt)

            for it in range(4):
                _attnv_group(7, it, pesb, ppos)
            _norm_gelu(7, 0, ppos, prcp)
            for it in range(4, 8):
                _attnv_group(7, it, pesb, ppos)
                _drain_it(it - 4)
            _norm_gelu(7, 1, ppos, prcp)
            for it in range(4, 8):
                _drain_it(it)

    _split_excess_waits(nc)
    return nc


def _split_excess_waits(nc):
    """walrus rejects >1 sem-wait per instruction ("Too many sync wait
    commands"); unroll extras into a chain of single-wait same-engine
    NoOps directly before the instruction."""
    ctr = 0
    for fn in nc.m.functions:
        for blk in fn.blocks:
            out = []
            for inst in blk.instructions:
                si = inst.sync_info
                if si is not None and len(si.on_wait) > 1:
                    for w in si.on_wait[:-1]:
                        nop = mybir.InstNoOp(name=f"waitnop-{ctr}")
                        ctr += 1
                        nop.engine = inst.engine
                        nop.sync_info = mybir.SyncInfo(
                            on_wait=[w], on_update=[])
                        out.append(nop)
                    inst.sync_info = mybir.SyncInfo(
                        on_wait=[si.on_wait[-1]], on_update=list(si.on_update))
                out.append(inst)
            blk.instructions = out


_NC_CACHE = None


def kernel(**inputs) -> np.ndarray:
    global _NC_CACHE
    x = np.ascontiguousarray(inputs["x"], dtype=np.float32)        # (8,32,32,256)
    shared = {
        "wq": np.ascontiguousarray(inputs["Wq"], dtype=np.float32),
        "wk": np.ascontiguousarray(inputs["Wk"], dtype=np.float32),
        "wv": np.ascontiguousarray(inputs["Wv"], dtype=np.float32),
        "wo": np.ascontiguousarray(inputs["Wo"], dtype=np.float32),
        "pe": np.ascontiguousarray(inputs["pos_emb"], dtype=np.float32),
        "bo": np.ascontiguousarray(inputs["bo"], dtype=np.float32),
        "gam": np.ascontiguousarray(inputs["gamma"], dtype=np.float32),
        "bet": np.ascontiguousarray(inputs["beta"], dtype=np.float32),
    }
    in_maps = []
    for c in range(NCORES):
        m = dict(shared)
        m["x"] = np.ascontiguousarray(x[c].reshape(N, D))
        in_maps.append(m)

    if _NC_CACHE is None:
        _NC_CACHE = build_nc()
    res = run_bass_kernel_spmd(_NC_CACHE, in_maps, core_ids=list(range(NCORES)))
    outs = [res.results[c]["out"].reshape(FM, FM, DOUT) for c in range(NCORES)]
    return np.stack(outs, axis=0)


if __name__ == "__main__":
    build_nc()
    print("build ok")



# revision 23
# speedup vs baseline: 1.1443x; 1.1443x over previous
"""Trainium2 Bass kernel for nn_Attention_89172110999574.

Strategy (8 NeuronCores, data parallel - 1 batch element per core):
  - All big matmuls run with f32r operands: the PE processes f32r moving
    data at 1 row/cycle when the moving free dim is >= 256, so weights/x
    stay f32 (no bf16 conversion pass) and precision improves.
  - PE transposes use a bf16 identity as the moving operand (cost keys on
    the moving dtype), so f32 data transposes at 1 row/cycle.
  - Scores computed TRANSPOSED (ST[j,i] = k_j . q_i); softmax scale folded
    into the q PSUM->SBUF copy.
  - Relative-position bias is block-Toeplitz; ONE per-head strip table is
    built once via large-elem DMAs (DRAM bounce for the partition
    reshuffle): msa (int16) holds bias*A + B Schraudolph addends.  Its
    bf16 BITCAST is simultaneously ~exp(bias) (Schraudolph identity), so
    the multiplicative table for the exact-exp path comes free.
  - exp(s+b) extraction is split per head across all three elementwise
    engines (the aggregate extraction rate is the real bottleneck):
      * A-tiles: ACT exp (PSUM->SBUF bf16) + DVE scalar_tensor_tensor
        multiply by the bf16 strip (all-SBUF 16-bit -> 4x DVE rate).
      * D-tiles: DVE scalar_tensor_tensor Schraudolph (one op from PSUM).
      * P-tiles: Pool scalar_tensor_tensor Schraudolph.
  - attn@V uses exp-scores as stationary -> [i, dv+1] tiles with the
    softmax denominator in the last column; 8 consecutive matmuls per
    PSUM accumulation group.
  - normalize+gelu fused: ACT Gelu with per-partition scale = 1/den.
  - BatchNorm affine folded into Wo (one-time tensor_mul) and bias row;
    per-i-tile tail: transpose via PE, final contraction, add, store.
"""

import os
import sys

import numpy as np

for _p in ("/opt/trn_rl_repo", "/root/.axon_site/_ro/trn_rl_repo"):
    if os.path.isdir(_p) and _p not in sys.path:
        sys.path.insert(0, _p)

import concourse.bass as bass
import concourse.tile as tile
from concourse import mybir
from concourse.bass_utils import run_bass_kernel_spmd
from concourse.masks import make_identity

N = 1024          # tokens per batch (32*32)
D = 256           # model dim
H = 8             # heads
DK = 32           # head dim (qk)
DV = 64           # head dim (v)
DOUT = 256        # output dim
NCORES = 8
FM = 32           # fmap
SCALE = float(DK) ** -0.5          # 1/sqrt(32)
BN_C = float(1.0 / np.sqrt(1.0 + 1e-5))
SCH_A = float(2 ** 7 / np.log(2.0))        # Schraudolph bf16 scale
SCH_B = float(127 * 2 ** 7 - 7.4 + 0.5)    # bias - minimax + trunc comp
F32 = mybir.dt.float32
F32R = mybir.dt.float32r
BF16 = mybir.dt.bfloat16
I16 = mybir.dt.int16
AF = mybir.ActivationFunctionType
ALU = mybir.AluOpType

# extraction path per (jt, ic) half-tile:
#   A = ACT Schraudolph bits (s*A, int16) + Pool stt add of the bias
#       addend table (Pool cannot touch PSUM, so it gets SBUF adds);
#   D = DVE scalar_tensor_tensor fused Schraudolph straight from PSUM.
A_HALF = {(0, 0), (0, 1), (1, 0), (1, 1), (2, 0), (2, 1), (3, 0)}
# strip-table u range covers all jt: u0 = 31-4*jt, slice [u0, u0+32)
MSA_U0, MSA_UN = 3, 60


def build_nc():
    nc = bass.Bass("TRN2", target_bir_lowering=False, debug=False)

    x = nc.dram_tensor("x", [N, D], F32, kind="ExternalInput").ap()
    wq = nc.dram_tensor("wq", [D, H * DK], F32, kind="ExternalInput").ap()
    wk = nc.dram_tensor("wk", [D, H * DK], F32, kind="ExternalInput").ap()
    wv = nc.dram_tensor("wv", [D, H * DV], F32, kind="ExternalInput").ap()
    wo = nc.dram_tensor("wo", [H * DV, DOUT], F32, kind="ExternalInput").ap()
    pe = nc.dram_tensor("pe", [N, H], F32, kind="ExternalInput").ap()
    bo = nc.dram_tensor("bo", [DOUT], F32, kind="ExternalInput").ap()
    gam = nc.dram_tensor("gam", [DOUT], F32, kind="ExternalInput").ap()
    bet = nc.dram_tensor("bet", [DOUT], F32, kind="ExternalInput").ap()
    out = nc.dram_tensor("out", [N, DOUT], F32, kind="ExternalOutput").ap()

    with tile.TileContext(nc) as tc:
        with (
            tc.tile_pool(name="const", bufs=1) as constp,
            tc.tile_pool(name="big", bufs=1) as bigp,
            tc.tile_pool(name="exps", bufs=3) as expp,
            tc.tile_pool(name="esb", bufs=1) as esbp,
            tc.tile_pool(name="small", bufs=2) as smallp,
            tc.tile_pool(name="yout", bufs=3) as youtp,
            tc.tile_pool(name="ps1", bufs=2, space="PSUM") as ps1p,
            tc.tile_pool(name="st", bufs=2, space="PSUM") as ps2p,
            tc.tile_pool(name="po", bufs=1, space="PSUM") as pop,
        ):
            # ------------- input / weight DMAs first (fabric order) -----
            # pe table first: the strip-table chain (engine ops + bounce +
            # gather + expansion) is the long pole gating head-0 extraction
            e_sb = smallp.tile([32, 32, 8], F32, tag="e_sb")
            nc.sync.dma_start(
                out=e_sb,
                in_=bass.AP(tensor=pe.tensor, offset=0,
                            ap=[[32 * H, 32], [H, 32], [1, 32 * H // 32]]),
            )
            xa = bigp.tile([128, 8, 256], F32R)
            for c in range(4):
                nc.sync.dma_start(
                    out=xa[:, 2 * c:2 * (c + 1), :],
                    in_=bass.AP(tensor=x.tensor, offset=2 * c * 128 * 256,
                                ap=[[256, 128], [128 * 256, 2], [1, 256]])
                    .bitcast(F32R))
            wq_sb = constp.tile([128, 2, 256], F32R)
            nc.scalar.dma_start(
                out=wq_sb,
                in_=bass.AP(tensor=wq.tensor, offset=0,
                            ap=[[256, 128], [128 * 256, 2], [1, 256]])
                .bitcast(F32R))
            wk_sb = constp.tile([128, 2, 256], F32R)
            nc.scalar.dma_start(
                out=wk_sb,
                in_=bass.AP(tensor=wk.tensor, offset=0,
                            ap=[[256, 128], [128 * 256, 2], [1, 256]])
                .bitcast(F32R))

            # ---------------- constants -------------------------------
            identb = constp.tile([128, 128], BF16)
            make_identity(nc, identb)
            identf = constp.tile([128, 128], F32R)
            nc.gpsimd.tensor_copy(identf, identb)

            # ---------------- strip table (Schraudolph addends) --------
            # eea = pe/scale*A + B int16;  bitcast-bf16(eea) ~ exp(pe/scale)
            eea = smallp.tile([32, 32, 8], I16, tag="eea")
            nc.scalar.activation(eea, e_sb, AF.Copy,
                                 scale=SCH_A / SCALE, bias=SCH_B)
            # s-flip: wrowa[a, h, s] = eea[a, |s-31|, h]
            wrowa = smallp.tile([32, 8, 63], I16, tag="wrowa")
            nc.gpsimd.tensor_copy(
                wrowa[:, :, 0:31],
                bass.AP(tensor=eea.tensor, offset=eea.offset + 31 * 8,
                        ap=[eea.ap[0], [1, 8], [-8, 31]]),
            )
            nc.gpsimd.tensor_copy(
                wrowa[:, :, 31:63],
                bass.AP(tensor=eea.tensor, offset=eea.offset,
                        ap=[eea.ap[0], [1, 8], [8, 32]]),
            )
            # gather straight from SBUF (DMA descriptors cross partitions,
            # so no DRAM bounce needed; wrowa palindromic in s, so the
            # gathered ci axis comes out reversed)
            # V weights must beat the table expansion to the DMA queue:
            # the V projection needs them at ~12us
            wv_sb = constp.tile([128, 2, 512], F32R)
            nc.sync.dma_start(
                out=wv_sb,
                in_=bass.AP(tensor=wv.tensor, offset=0,
                            ap=[[512, 128], [128 * 512, 2], [1, 512]])
                .bitcast(F32R))
            # msa[(g,cj), u-U0, h, ci'] = tab_h[|u-31-g|, |ci-cj|]
            msa = bigp.tile([128, MSA_UN, H, 32], I16)
            with tc.tile_pool(name="uw", bufs=1) as uwp:
                uwsba = uwp.tile([32, 32, H, 32], I16)
                nc.scalar.dma_start(
                    out=uwsba,
                    in_=bass.AP(tensor=wrowa.tensor, offset=wrowa.offset,
                                ap=[[1, 32], [504, 32], [63, 8], [1, 32]]),
                )
                # u-expansion: dst[(g,cj), u] = uwsba[cj, |u-31-g|]
                engs = (nc.sync, nc.scalar, nc.gpsimd)
                ei = 0
                for g in range(4):
                    # upper: u in [31+g, U0+UN), a = u-31-g ascending
                    ua, ub = 31 + g, MSA_U0 + MSA_UN
                    engs[ei % 3].dma_start(
                        out=msa[32 * g:32 * (g + 1),
                                ua - MSA_U0:ub - MSA_U0, :, :],
                        in_=uwsba[:, 0:ub - ua, :, :],
                    )
                    ei += 1
                    # lower: u in [U0, 31+g), a = 31+g-u descending
                    la, lb = MSA_U0, 31 + g
                    amax = 31 + g - MSA_U0
                    engs[ei % 3].dma_start(
                        out=msa[32 * g:32 * (g + 1), 0:lb - la, :, :],
                        in_=bass.AP(tensor=uwsba.tensor,
                                    offset=uwsba.offset + amax * 256,
                                    ap=[uwsba.ap[0], [-256, lb - la],
                                        [1, 256]]),
                    )
                    ei += 1

            # late loads (needed only for the drain)
            g2b = constp.tile([128, DOUT], F32)
            b2b = constp.tile([128, DOUT], F32)
            tmpb = constp.tile([128, DOUT], F32)
            nc.sync.dma_start(
                out=g2b, in_=bass.AP(tensor=gam.tensor, offset=0,
                                     ap=[[0, 128], [1, DOUT]]))
            nc.sync.dma_start(
                out=b2b, in_=bass.AP(tensor=bet.tensor, offset=0,
                                     ap=[[0, 128], [1, DOUT]]))
            nc.sync.dma_start(
                out=tmpb, in_=bass.AP(tensor=bo.tensor, offset=0,
                                      ap=[[0, 128], [1, DOUT]]))
            wo_sb = constp.tile([128, 4, 256], F32)
            nc.sync.dma_start(
                out=wo_sb,
                in_=bass.AP(tensor=wo.tensor, offset=0,
                            ap=[[256, 128], [128 * 256, 4], [1, 256]]))
            wo_b = constp.tile([128, 4, 256], BF16)

            def _copy3(i, dst, src, scale=None):
                # PSUM sources: ACT/DVE only (Pool cannot access PSUM)
                if i % 2 == 0:
                    if scale is None:
                        nc.scalar.copy(dst, src)
                    else:
                        nc.scalar.mul(dst, src, scale)
                else:
                    if scale is None:
                        nc.vector.tensor_copy(dst, src)
                    else:
                        nc.vector.tensor_scalar_mul(dst, src, scale)

            # ---------------- phase A: x -> xT (f32, via PE) ------------
            xT = bigp.tile([128, 2, N], F32R)

            def _phase_a(nts):
                for nt in nts:
                    pst = ps1p.tile([128, 512], F32, tag="ps1")
                    for dt in range(2):
                        nc.tensor.matmul(
                            pst[:, 128 * dt:128 * (dt + 1)].bitcast(F32R),
                            xa[:, nt, 128 * dt:128 * (dt + 1)],
                            identf, is_transpose=True)
                    _copy3(nt,
                           bass.AP(tensor=xT.tensor,
                                   offset=xT.offset + 128 * nt,
                                   ap=[xT.ap[0], [N, 2], [1, 128]]),
                           pst[:, 0:256].rearrange("p (d c) -> p d c", c=128))

            qT = bigp.tile([128, 2, N], F32R)
            kT = bigp.tile([128, 2, N], F32R)
            va = bigp.tile([128, 8, H, 65], BF16)
            nc.scalar.activation(va[:, :, :, 64:65], identb[:, 0:64],
                                 AF.Copy, bias=1.0, scale=0.0)
            ci_ = 0

            def _qk(mt, ics=(0, 1)):
                nonlocal ci_
                for dst_sb, w_sb, scl in ((qT, wq_sb, SCALE),
                                          (kT, wk_sb, None)):
                    for ic in ics:
                        ps = ps1p.tile([128, 512], F32, tag="ps1")
                        for kt in range(2):
                            nc.tensor.matmul(
                                ps,
                                w_sb[:, kt, 128 * mt:128 * (mt + 1)],
                                xT[:, kt, 512 * ic:512 * (ic + 1)],
                                start=(kt == 0), stop=(kt == 1),
                            )
                        _copy3(ci_, dst_sb[:, mt, 512 * ic:512 * (ic + 1)],
                               ps, scale=scl)
                        ci_ += 1

            # tokens 0-511 transposed first -> q/k ic=0 can start at once
            _phase_a(range(4))
            _qk(0, ics=(0,))
            _phase_a(range(4, 8))
            _qk(0, ics=(1,))
            _qk(1)
            # V projection last: wv rides the DMA queue behind the
            # strip-table gather, so q/k work fills that window
            for jt in range(8):
                ps = ps1p.tile([128, 512], F32, tag="ps1")
                for kt in range(2):
                    nc.tensor.matmul(
                        ps,
                        xT[:, kt, 128 * jt:128 * (jt + 1)],
                        wv_sb[:, kt, :],
                        start=(kt == 0), stop=(kt == 1),
                    )
                psr = ps.rearrange("p (h v) -> p h v", v=64)
                _copy3(ci_, va[:, jt, :, 0:64], psr)
                ci_ += 1

            # one-time BN affine folds (emitted late: their engine ops must
            # not delay the startup-critical ACT/DVE queues)
            nc.scalar.mul(g2b, g2b, BN_C)
            nc.vector.tensor_mul(tmpb, tmpb, g2b)
            nc.vector.tensor_add(b2b, b2b, tmpb)
            nc.vector.tensor_mul(
                wo_b, wo_sb,
                bass.AP(tensor=g2b.tensor, offset=g2b.offset,
                        ap=[g2b.ap[0], [0, 4], [1, 256]]))

            # ---------------- phase C: attention -----------------------
            # g_all[i-part, blk, it, h%2, dv] collects gelu(attn/den);
            # blk = head pair. Layout keeps each blk contiguous so ONE
            # DMA-XBAR transpose per blk produces gtt[hv, it, i] directly.
            g_all = bigp.tile([128, 4, 8, 2, DV], BF16)
            gtt = bigp.tile([128, 4, 8, 128], BF16)

            def _tail_transpose(blk, its=range(8)):
                its = list(its)
                nc.sync.dma_start(
                    out=gtt[:, blk, its[0]:its[-1] + 1, :],
                    in_=bass.AP(
                        tensor=g_all.tensor,
                        offset=g_all.offset + blk * 1024 + its[0] * 128,
                        ap=[g_all.ap[0], [1, 128 * len(its)]]),
                    transpose=True)

            def _attnv_group(h, it, esbbs, pos):
                for jt in range(8):
                    nc.tensor.matmul(
                        pos[it // 4][:, it % 4, :],
                        esbbs[jt][:, 128 * it:128 * (it + 1)],
                        va[:, jt, h, :],
                        start=(jt == 0), stop=(jt == 7),
                    )

            def _norm_gelu(h, half, pos, rcp):
                den = pos[half][:, :, 64:65]
                nc.vector.reciprocal(
                    rcp[:, 4 * half:4 * half + 4],
                    bass.AP(tensor=den.tensor, offset=den.offset,
                            ap=[den.ap[0], [65, 4]]))
                for it in range(4 * half, 4 * half + 4):
                    nc.scalar.activation(
                        g_all[:, h // 2, it, h % 2, :],
                        pos[half][:, it % 4, 0:64],
                        AF.Gelu, scale=rcp[:, it:it + 1])

            def _msr(h, jt, dtype, ic=None):
                u0 = 31 - 4 * jt
                if ic is None:
                    msl = msa[:, u0 - MSA_U0:u0 - MSA_U0 + 32, h, :]
                else:
                    msl = msa[:, u0 - MSA_U0 + 16 * ic:
                              u0 - MSA_U0 + 16 * (ic + 1), h, :]
                ap = bass.AP(tensor=msl.tensor, offset=msl.offset + 31,
                             ap=[msl.ap[0], msl.ap[1], [-1, 32]])
                return ap if dtype is I16 else ap.bitcast(BF16)

            # attnV/normalize are software-pipelined one head behind the
            # score/exp stream so the attnV matmul groups fill PE stalls
            # between score matmuls (keeps PE p-state ramped).
            prev = None
            # alternate ACT-exp (A) and DVE-stt (D) tiles so both
            # extraction engines stream concurrently
            jt_order = (0, 4, 1, 5, 2, 6, 3, 7)
            for h in range(H):
                mtk = h // 4
                pb = 32 * (h % 4)
                po0 = pop.tile([128, 4, 65], F32, tag="po0")
                po1 = pop.tile([128, 4, 65], F32, tag="po1")
                pos = (po0, po1)
                esbbs = [None] * 8
                for step, jt in enumerate(jt_order):
                    esb = esbp.tile([128, 1024], I16, tag="esbi", bufs=16)
                    esbbs[jt] = esb.bitcast(BF16)
                    for ic in range(2):
                        psp = ps2p if (2 * step + ic) % 3 != 2 else ps1p
                        tg = "st" if psp is ps2p else "ps1"
                        ps = psp.tile([128, 512], F32, tag=tg,
                                      bufs=(4 if psp is ps2p else 2))
                        nc.tensor.matmul(
                            ps,
                            kT[pb:pb + 32, mtk, 128 * jt:128 * (jt + 1)],
                            qT[pb:pb + 32, mtk, 512 * ic:512 * (ic + 1)],
                            start=True, stop=True,
                            tile_position=(pb, 0),
                        )
                        sl = slice(512 * ic, 512 * (ic + 1))
                        if (jt, ic) in A_HALF:
                            # exact exp on ACT (bf16); multiplicative bias
                            # on Pool (float-only engine; the bf16 bitcast
                            # of msa ~ exp(bias) by the Schraudolph
                            # identity), one half on DVE (2x packed rate)
                            es = expp.tile([128, 512], BF16, tag="es",
                                           bufs=6)
                            nc.scalar.activation(es, ps, AF.Exp)
                            meng = nc.vector if jt == 3 else nc.gpsimd
                            meng.tensor_tensor(
                                esb.bitcast(BF16)[:, sl], es,
                                _msr(h, jt, BF16, ic), ALU.mult)
                        else:
                            # fused Schraudolph exp+bias on DVE:
                            # bits_i16 = ps*A + (b*A + B) -> bitcast bf16
                            nc.vector.scalar_tensor_tensor(
                                esb[:, sl], ps, SCH_A,
                                _msr(h, jt, I16, ic),
                                ALU.mult, ALU.add)
                    # head h-1's attnV/normalize, software-pipelined two
                    # steps behind this head's score/exp stream so the last
                    # extraction tiles of h-1 are ready before PE needs them
                    if prev is not None:
                        pesb, ppos, prcp = prev
                        if step in (2, 3, 4, 5):
                            _attnv_group(h - 1, 2 * step - 4, pesb, ppos)
                            _attnv_group(h - 1, 2 * step - 3, pesb, ppos)
                        elif step == 7:
                            _norm_gelu(h - 1, 0, ppos, prcp)
                            _norm_gelu(h - 1, 1, ppos, prcp)
                            if h % 2 == 0:
                                _tail_transpose(h // 2 - 1)
                rcp = smallp.tile([128, 8], F32, tag="rcp", bufs=3)
                prev = (esbbs, pos, rcp)
            # drain last head
            pesb, ppos, prcp = prev

            def _drain_it(it):
                ps = ps1p.tile([128, 512], F32, tag="ps1")
                for kt in range(4):
                    nc.tensor.matmul(
                        ps[:, 0:256],
                        gtt[:, kt, it, :],
                        wo_b[:, kt, :],
                        start=(kt == 0), stop=(kt == 3),
                    )
                yt = youtp.tile([128, DOUT], F32, tag="yt", bufs=8)
                nc.vector.tensor_add(yt, ps[:, 0:256], b2b)
                eng = nc.sync if it % 2 == 0 else nc.scalar
                eng.dma_start(out=out[128 * it:128 * (it + 1), :], in_=yt)

            for it in range(4):
                _attnv_group(7, it, pesb, ppos)
            _norm_gelu(7, 0, ppos, prcp)
            _tail_transpose(3, its=range(0, 4))
            for it in range(4, 8):
                _attnv_group(7, it, pesb, ppos)
                _drain_it(it - 4)
            _norm_gelu(7, 1, ppos, prcp)
            _tail_transpose(3, its=range(4, 8))
            for it in range(4, 8):
                _drain_it(it)

    _split_excess_waits(nc)
    return nc


def _split_excess_waits(nc):
    """walrus rejects >1 sem-wait per instruction ("Too many sync wait
    commands"); unroll extras into a chain of single-wait same-engine
    NoOps directly before the instruction."""
    ctr = 0
    for fn in nc.m.functions:
        for blk in fn.blocks:
            out = []
            for inst in blk.instructions:
                si = inst.sync_info
                if si is not None and len(si.on_wait) > 1:
                    for w in si.on_wait[:-1]:
                        nop = mybir.InstNoOp(name=f"waitnop-{ctr}")
                        ctr += 1
                        nop.engine = inst.engine
                        nop.sync_info = mybir.SyncInfo(
                            on_wait=[w], on_update=[])
                        out.append(nop)
                    inst.sync_info = mybir.SyncInfo(
                        on_wait=[si.on_wait[-1]], on_update=list(si.on_update))
                out.append(inst)
            blk.instructions = out


_NC_CACHE = None


def kernel(**inputs) -> np.ndarray:
    global _NC_CACHE
    x = np.ascontiguousarray(inputs["x"], dtype=np.float32)        # (8,32,32,256)
    shared = {
        "wq": np.ascontiguousarray(inputs["Wq"], dtype=np.float32),
        "wk": np.ascontiguousarray(inputs["Wk"], dtype=np.float32),
        "wv": np.ascontiguousarray(inputs["Wv"], dtype=np.float32),
        "wo": np.ascontiguousarray(inputs["Wo"], dtype=np.float32),
        "pe": np.ascontiguousarray(inputs["pos_emb"], dtype=np.float32),
        "bo": np.ascontiguousarray(inputs["bo"], dtype=np.float32),
        "gam": np.ascontiguousarray(inputs["gamma"], dtype=np.float32),
        "bet": np.ascontiguousarray(inputs["beta"], dtype=np.float32),
    }
    in_maps = []
    for c in range(NCORES):
        m = dict(shared)
        m["x"] = np.ascontiguousarray(x[c].reshape(N, D))
        in_maps.append(m)

    if _NC_CACHE is None:
        _NC_CACHE = build_nc()
    res = run_bass_kernel_spmd(_NC_CACHE, in_maps, core_ids=list(range(NCORES)))
    outs = [res.results[c]["out"].reshape(FM, FM, DOUT) for c in range(NCORES)]
    return np.stack(outs, axis=0)


if __name__ == "__main__":
    build_nc()
    print("build ok")
